# revision 1
# baseline (speedup 1.0000x reference)
"""DeepSeek MoE layer on 8 Trainium2 NeuronCores (Bass/Tile).

Sharding: expert parallelism. Core c owns routed experts 4c..4c+3 and a
256-wide slice of the shared experts' intermediate dim. The gate (routing)
is replicated on every core. Each core produces a partial output in
transposed layout [H, T]; the host sums the 8 partials and transposes.

Layout strategy: the host pre-transposes x/gate_w (so no on-device
transposes are needed for the big operands) and re-tiles the expert
weights into [.., 128, 512] bf16 blocks so every weight DMA is one fully
contiguous 128KB burst. Expert matmuls run in bf16 (weights rounded once
on the host); the routing path (logits, combine weights) is computed in
exact fp32 so top-k selections match the reference.
"""

import sys

sys.path.insert(0, "/opt/trn_rl_repo")

import numpy as np
import ml_dtypes

import concourse.bass as bass  # noqa: F401
import concourse.mybir as mybir
import concourse.tile as tile
from concourse import bacc
from concourse.bass_utils import run_bass_kernel_spmd
from concourse.masks import make_identity

F32 = mybir.dt.float32
BF16 = mybir.dt.bfloat16
AF = mybir.ActivationFunctionType
ALU = mybir.AluOpType

# Problem constants (hardcoded per contract).
T = 512       # tokens
H = 2048      # hidden
I = 1024      # moe intermediate
I2 = 2 * I    # gate+up cols per expert
E = 32        # routed experts
K = 8         # experts per token
NG = 8        # routing groups
TG = 4        # top-k groups
SCALE = 2.5   # routed scaling factor
NCORES = 8
EL = E // NCORES          # local experts per core = 4
SI = 256                  # shared-intermediate slice per core (2*1024/8)
P = 128
HK = H // P               # 16 k-tiles over hidden
TM = T // P               # 4 token tiles
IK = I // P               # 8 k-tiles over intermediate
NEG1 = -1.0e30
NEG2 = -2.0e30


def _routing(tc, d, pools, ident, xT32):
    """Compute cwb [128, EL, T]: per-local-expert combine weights broadcast
    across partitions. Exact fp32 everywhere."""
    nc = tc.nc
    sb, work, stream, psA, psB = pools

    # gwT [128, HK, E] from host-pretransposed gate_w
    gwT = sb.tile([P, HK, E], F32, name="gwT")
    nc.sync.dma_start(gwT[:], d["gwT"].rearrange("(k p) e -> p k e", p=P))
    gbb = sb.tile([P, E], F32, name="gbb")
    nc.sync.dma_start(gbb[:], d["gbb"][:])
    neg = sb.tile([P, E], F32, name="neg")
    nc.vector.memset(neg[:], NEG1)

    # logitsT [E, T] = gate_w @ x.T   (fp32 matmul mode, exact)
    plgT = psB.tile([E, T], F32, tag="small")
    for k in range(HK):
        nc.tensor.matmul(plgT[:], gwT[:, k, :], xT32[:, k, :],
                         start=(k == 0), stop=(k == HK - 1))
    lgT = work.tile([E, T], F32, tag="lgT")
    nc.vector.tensor_copy(lgT[:], plgT[:])

    cw_all = sb.tile([P, TM, E], F32, name="cw_all")
    for m in range(TM):
        # logits tile [128 tok, E] via PE transpose
        plg = psB.tile([P, E], F32, tag="small")
        nc.tensor.transpose(plg[:], lgT[:, m * P:(m + 1) * P], ident[:E, :E])
        s_t = work.tile([P, E], F32, tag="s_t")
        nc.scalar.activation(s_t[:], plg[:], AF.Sigmoid)
        sc = work.tile([P, E], F32, tag="sc")
        nc.vector.tensor_add(sc[:], s_t[:], gbb[:])

        # group score: sum of top-2 within each group of 4
        sc3 = sc[:].rearrange("p (g f) -> p g f", f=4)
        ga = work.tile([P, NG], F32, tag="ga")
        gb_ = work.tile([P, NG], F32, tag="gb_")
        gc = work.tile([P, NG], F32, tag="gc")
        gd = work.tile([P, NG], F32, tag="gd")
        nc.vector.tensor_tensor(ga[:], sc3[:, :, 0], sc3[:, :, 1], ALU.max)
        nc.vector.tensor_tensor(gb_[:], sc3[:, :, 0], sc3[:, :, 1], ALU.min)
        nc.vector.tensor_tensor(gc[:], sc3[:, :, 2], sc3[:, :, 3], ALU.max)
        nc.vector.tensor_tensor(gd[:], sc3[:, :, 2], sc3[:, :, 3], ALU.min)
        hi = work.tile([P, NG], F32, tag="hi")
        lo = work.tile([P, NG], F32, tag="lo")
        mid = work.tile([P, NG], F32, tag="mid")
        nc.vector.tensor_tensor(hi[:], ga[:], gc[:], ALU.max)
        nc.vector.tensor_tensor(lo[:], ga[:], gc[:], ALU.min)
        nc.vector.tensor_tensor(mid[:], gb_[:], gd[:], ALU.max)
        gsc = work.tile([P, NG], F32, tag="gsc")
        nc.vector.tensor_tensor(gsc[:], lo[:], mid[:], ALU.max)
        nc.vector.tensor_add(gsc[:], gsc[:], hi[:])

        # top-TG groups -> 0/1 group mask
        gm8 = work.tile([P, 8], F32, tag="gm8")
        nc.vector.max(gm8[:], gsc[:])
        nc.vector.memset(gm8[:, TG:], NEG1)
        gz = work.tile([P, NG], F32, tag="gz")
        nc.vector.match_replace(out=gz[:], in_to_replace=gm8[:], in_values=gsc[:], imm_value=NEG1)
        gmask = work.tile([P, NG], mybir.dt.uint32, tag="gmask")
        nc.vector.tensor_scalar(gmask[:], gz[:], -5.0e29, None, op0=ALU.is_le)

        # expand to experts, mask scores
        emask = work.tile([P, E], mybir.dt.uint32, tag="emask")
        em3 = emask[:].rearrange("p (g f) -> p g f", f=4)
        nc.vector.tensor_copy(em3[:], gmask[:, :, None].to_broadcast([P, NG, 4]))
        msk = work.tile([P, E], F32, tag="msk")
        nc.vector.select(out=msk[:], mask=emask[:], on_true=sc[:], on_false=neg[:])

        # top-K experts -> 0/1 selection mask
        t8 = work.tile([P, 8], F32, tag="t8")
        nc.vector.max(t8[:], msk[:])
        mz = work.tile([P, E], F32, tag="mz")
        nc.vector.match_replace(out=mz[:], in_to_replace=t8[:], in_values=msk[:], imm_value=NEG2)
        sel = work.tile([P, E], F32, tag="selm")
        nc.vector.tensor_scalar(sel[:], mz[:], -1.5e30, None, op0=ALU.is_le)

        # weights: s * sel, renormalized, * SCALE
        wr = work.tile([P, E], F32, tag="wr")
        nc.vector.tensor_mul(wr[:], s_t[:], sel[:])
        ws = work.tile([P, 1], F32, tag="ws")
        nc.vector.reduce_sum(ws[:], wr[:], axis=mybir.AxisListType.X)
        rec = work.tile([P, 1], F32, tag="rec")
        nc.vector.reciprocal(rec[:], ws[:])
        coef = work.tile([P, 1], F32, tag="coef")
        nc.vector.tensor_scalar_mul(coef[:], rec[:], SCALE)
        nc.vector.tensor_scalar_mul(cw_all[:, m, :], wr[:], coef[:])
    return cw_all


def _routing_post(tc, d, pools, ident, cw_all):
    # transpose cw tiles and broadcast local experts' rows across partitions
    nc = tc.nc
    sb, work, stream, psA, psB = pools
    cwT = sb.tile([E, T], F32, name="cwT")
    for m in range(TM):
        ptc = psB.tile([E, P], F32, tag="small")
        nc.tensor.transpose(ptc[:], cw_all[:, m, :], ident[:])
        nc.vector.tensor_copy(cwT[:, m * P:(m + 1) * P], ptc[:])
    bselS = sb.tile([E, EL * P], F32, name="bselS")
    nc.sync.dma_start(bselS[:], d["bsel"][:])
    cwb = sb.tile([P, EL, T], F32, name="cwb")
    for j in range(EL):
        pb = psA.tile([P, T], F32, tag="mm", name=f"pcwb{j}")
        nc.tensor.matmul(pb[:], bselS[:, j * P:(j + 1) * P], cwT[:], start=True, stop=True)
        nc.vector.tensor_copy(cwb[:, j, :], pb[:])
    return cwb


def _build_body(tc, d, pools):
    nc = tc.nc
    sb, work, stream, psA, psB = pools

    ident = sb.tile([P, P], F32, name="ident")
    make_identity(nc, ident)

    # x arrives pre-transposed from host: xT [H, T] fp32 and bf16
    xT32 = sb.tile([P, HK, T], F32, name="xT32", tag="big32")
    xTb = sb.tile([P, HK, T], BF16, name="xTb")
    xr = d["xT"].rearrange("(k p) t -> p k t", p=P)
    xbr = d["xTb"].rearrange("(k p) t -> p k t", p=P)
    for k in range(HK):
        nc.sync.dma_start(xT32[:, k, :], xr[:, k, :])
        nc.sync.dma_start(xTb[:, k, :], xbr[:, k, :])

    cw_all = _routing(tc, d, pools, ident, xT32)

    # ---- shared experts gate_up first (acts needed in fused down phase) ----
    pss = [psA.tile([P, T], F32, tag="mm", name=f"pss{i}") for i in range(4)]
    for k in range(HK):
        sws = stream.tile([P, 512], BF16, tag="wstream")
        nc.sync.dma_start(sws[:], d["swgu"][k, :, :])
        for i in range(4):
            nc.tensor.matmul(pss[i][:], sws[:, i * P:(i + 1) * P], xTb[:, k, :],
                             start=(k == 0), stop=(k == HK - 1))
    acts = work.tile([P, 2, T], BF16, tag="acts")
    for t in range(2):
        sst = work.tile([P, T], F32, tag="sst")
        nc.scalar.activation(sst[:], pss[t][:], AF.Sigmoid)
        nc.vector.tensor_mul(sst[:], sst[:], pss[t][:])
        nc.vector.tensor_mul(acts[:, t, :], sst[:], pss[2 + t][:])

    # ---- routed experts gate_up -> actw[j] (bf16, [128, IK, T] each) ----
    actws = []
    cwb = None
    for j in range(EL):
        sg = work.tile([P, IK, T], F32, tag="sg")
        actw = sb.tile([P, IK, T], BF16, tag=f"actw{j}", name=f"actw{j}")
        actws.append(actw)
        for q in range(4):
            if j == 0 and q == 2:
                cwb = _routing_post(tc, d, pools, ident, cw_all)
            pps = [psA.tile([P, T], F32, tag="mm", name=f"pps{i}") for i in range(4)]
            for k in range(HK):
                wst = stream.tile([P, 512], BF16, tag="wstream")
                nc.sync.dma_start(wst[:], d["wgu"][j, q, k, :, :])
                for i in range(4):
                    nc.tensor.matmul(pps[i][:], wst[:, i * P:(i + 1) * P], xTb[:, k, :],
                                     start=(k == 0), stop=(k == HK - 1))
            if q < 2:
                for i in range(4):
                    it = 4 * q + i
                    sgm = work.tile([P, T], F32, tag="sgm")
                    nc.scalar.activation(sgm[:], pps[i][:], AF.Sigmoid)
                    nc.vector.tensor_mul(sg[:, it, :], sgm[:], pps[i][:])
            else:
                for i in range(4):
                    it = 4 * (q - 2) + i
                    atmp = work.tile([P, T], F32, tag="atmp")
                    nc.vector.tensor_mul(atmp[:], sg[:, it, :], pps[i][:])
                    nc.vector.tensor_mul(actw[:, it, :], atmp[:], cwb[:, j, :])

    # ---- fused down phase: all 4 experts + shared accumulate in PSUM ----
    outT = sb.tile([P, HK, T], F32, name="outT", tag="big32")
    for hq in range(4):
        ppd = [psA.tile([P, T], F32, tag="mm", name=f"ppd{i}") for i in range(4)]
        for j in range(EL):
            for i2 in range(IK):
                wds = stream.tile([P, 512], BF16, tag="wstream")
                nc.sync.dma_start(wds[:], d["wd"][hq, j, i2, :, :])
                for h in range(4):
                    nc.tensor.matmul(ppd[h][:], wds[:, h * P:(h + 1) * P],
                                     actws[j][:, i2, :],
                                     start=(j == 0 and i2 == 0), stop=False)
        for i2 in range(2):
            wds = stream.tile([P, 512], BF16, tag="wstream")
            nc.sync.dma_start(wds[:], d["swd"][hq, i2, :, :])
            for h in range(4):
                nc.tensor.matmul(ppd[h][:], wds[:, h * P:(h + 1) * P],
                                 acts[:, i2, :],
                                 start=False, stop=(i2 == 1))
        for h in range(4):
            nc.vector.tensor_copy(outT[:, 4 * hq + h, :], ppd[h][:])
        nc.sync.dma_start(
            d["outT"].rearrange("(ho p) t -> p ho t", p=P)[:, 4 * hq:4 * hq + 4, :],
            outT[:, 4 * hq:4 * hq + 4, :])


def build_nc(repeat=1):
    nc = bacc.Bacc("TRN2", target_bir_lowering=False, debug=False, num_devices=NCORES)
    d = {
        "xT": nc.dram_tensor("xT", [H, T], F32, kind="ExternalInput").ap(),
        "xTb": nc.dram_tensor("xTb", [H, T], BF16, kind="ExternalInput").ap(),
        "gwT": nc.dram_tensor("gwT", [H, E], F32, kind="ExternalInput").ap(),
        "gbb": nc.dram_tensor("gbb", [P, E], F32, kind="ExternalInput").ap(),
        "bsel": nc.dram_tensor("bsel", [E, EL * P], F32, kind="ExternalInput").ap(),
        "wgu": nc.dram_tensor("wgu", [EL, 4, HK, P, 512], BF16, kind="ExternalInput").ap(),
        "wd": nc.dram_tensor("wd", [4, EL, IK, P, 512], BF16, kind="ExternalInput").ap(),
        "swgu": nc.dram_tensor("swgu", [HK, P, 512], BF16, kind="ExternalInput").ap(),
        "swd": nc.dram_tensor("swd", [4, 2, P, 512], BF16, kind="ExternalInput").ap(),
        "outT": nc.dram_tensor("outT", [H, T], F32, kind="ExternalOutput").ap(),
    }
    with tile.TileContext(nc) as tc:
        with (
            tc.tile_pool(name="sb", bufs=1) as sb,
            tc.tile_pool(name="work", bufs=2) as work,
            tc.tile_pool(name="stream", bufs=6) as stream,
            tc.tile_pool(name="psA", bufs=6, space="PSUM") as psA,
            tc.tile_pool(name="psB", bufs=2, space="PSUM") as psB,
        ):
            pools = (sb, work, stream, psA, psB)
            if repeat == 1:
                _build_body(tc, d, pools)
            else:
                with tc.For_i(0, repeat, 1):
                    _build_body(tc, d, pools)
    nc.compile()
    return nc


def shard_inputs(hidden_states, gate_w, gate_bias, w_gate_up, w_down,
                 shared_w_gate_up, shared_w_down):
    bf = ml_dtypes.bfloat16
    x = np.ascontiguousarray(hidden_states, dtype=np.float32)
    xT = np.ascontiguousarray(x.T)
    xTb = np.ascontiguousarray(xT.astype(bf))
    gwT = np.ascontiguousarray(np.asarray(gate_w, np.float32).T)
    gbb = np.ascontiguousarray(np.tile(np.asarray(gate_bias, np.float32)[None, :], (P, 1)))
    wgu = np.asarray(w_gate_up, np.float32)
    wd = np.asarray(w_down, np.float32)
    swgu = np.asarray(shared_w_gate_up, np.float32)
    swd = np.asarray(shared_w_down, np.float32)

    in_maps = []
    for c in range(NCORES):
        bsel = np.zeros((E, EL * P), dtype=np.float32)
        for j in range(EL):
            bsel[EL * c + j, j * P:(j + 1) * P] = 1.0
        # wgu_r[j, q, k] = wgu[e][k*128:(k+1)*128, q*512:(q+1)*512]
        wgu_c = wgu[EL * c:EL * (c + 1)].astype(bf)
        wgu_r = np.ascontiguousarray(
            wgu_c.reshape(EL, HK, P, 4, 512).transpose(0, 3, 1, 2, 4))
        # wd_r[hq, j, i2] = wd[e][i2*128:(i2+1)*128, hq*512:(hq+1)*512]
        wd_c = wd[EL * c:EL * (c + 1)].astype(bf)
        wd_r = np.ascontiguousarray(
            wd_c.reshape(EL, IK, P, 4, 512).transpose(3, 0, 1, 2, 4))
        # shared: pack [gate slice | up slice] columns -> [H, 512] -> [HK, 128, 512]
        sw = np.concatenate([
            swgu[:, c * SI:(c + 1) * SI],
            swgu[:, 2 * I + c * SI: 2 * I + (c + 1) * SI],
        ], axis=1).astype(bf)
        swgu_r = np.ascontiguousarray(sw.reshape(HK, P, 512))
        # swd_r[hq, i2] = swd_slice[i2*128:(i2+1)*128, hq*512:(hq+1)*512]
        sd = swd[c * SI:(c + 1) * SI, :].astype(bf)
        swd_r = np.ascontiguousarray(sd.reshape(2, P, 4, 512).transpose(2, 0, 1, 3))
        in_maps.append({
            "xT": xT, "xTb": xTb, "gwT": gwT, "gbb": gbb, "bsel": bsel,
            "wgu": wgu_r, "wd": wd_r, "swgu": swgu_r, "swd": swd_r,
        })
    return in_maps


_NC_CACHE = {}


def kernel(hidden_states, gate_w, gate_bias, w_gate_up, w_down,
           shared_w_gate_up, shared_w_down):
    if "nc" not in _NC_CACHE:
        _NC_CACHE["nc"] = build_nc(repeat=1)
    nc = _NC_CACHE["nc"]
    in_maps = shard_inputs(hidden_states, gate_w, gate_bias, w_gate_up, w_down,
                           shared_w_gate_up, shared_w_down)
    res = run_bass_kernel_spmd(nc, in_maps, list(range(NCORES)))
    acc = np.zeros((H, T), dtype=np.float32)
    for c in range(NCORES):
        acc += res.results[c]["outT"]
    return np.ascontiguousarray(acc.T)



# revision 5
# speedup vs baseline: 1.8628x; 1.8628x over previous
"""DeepSeek MoE layer on 8 Trainium2 NeuronCores (Bass/Tile) - sparse dispatch.

Sharding: expert parallelism. Core c owns routed experts 4c..4c+3 and a
256-wide slice of the shared experts' intermediate dim. Routing is
replicated on every core (exact fp32). Instead of computing every expert
densely over all 512 tokens, each core compacts the ~128-168 tokens routed
to each local expert into a per-expert batch via one-hot gather matmuls,
runs the expert MLP on the compacted batch, and scatter-adds the results
back to dense token space with combine weights folded into the scatter
matrix. Per-core output is a dense [T, H] fp32 partial; the host sums the
8 partials.

On-device pipeline per core:
  1. logits = gate_w @ x (fp32), sigmoid scores, grouped top-k -> combine
     weights cw[t,e] (fp32, 0 for unselected) - same as reference.
  2. prefix[t,e] = inclusive count of selections of expert e among tokens
     <= t, via matmuls with ones/upper-triangular lhsT.
  3. Per local expert j: gather matrix Sg[t,slot] = (prefix==slot+1)&sel
     (bf16 one-hot) and weighted scatter matrix SwT[slot,t] =
     (prefix==slot+1) * cw (bf16), built with DVE compares.
  4. x_g[h, slot] = x^T @ Sg  (gather matmul, bf16)
  5. gu = wgu^T @ x_g ; act = silu(gate)*up  per expert (bf16)
  6. y[slot, h] = act-chunks^T @ wd (token-partitioned down matmul)
  7. out[t, h] += SwT_j^T @ y_j for each expert + shared-expert down,
     accumulated in PSUM over a token-tile sweep.
Capacity per expert is uniform (C, derived from the input's routing on
host, rounded up; slots beyond an expert's count contribute zero).
"""

import sys

sys.path.insert(0, "/opt/trn_rl_repo")

import numpy as np
import ml_dtypes

import concourse.bass as bass  # noqa: F401
import concourse.mybir as mybir
import concourse.tile as tile
from concourse import bacc
from concourse.bass_utils import run_bass_kernel_spmd
from concourse.masks import make_identity, make_upper_triangular

F32 = mybir.dt.float32
BF16 = mybir.dt.bfloat16
I32 = mybir.dt.int32
AF = mybir.ActivationFunctionType
ALU = mybir.AluOpType

# Problem constants (hardcoded per contract).
T = 512       # tokens
H = 2048      # hidden
I = 1024      # moe intermediate
E = 32        # routed experts
K = 8         # experts per token
NG = 8        # routing groups
TG = 4        # top-k groups
SCALE = 2.5   # routed scaling factor
NCORES = 8
EL = E // NCORES          # local experts per core = 4
SI = 256                  # shared-intermediate slice per core
P = 128
HK = H // P               # 16 k-tiles over hidden
TM = T // P               # 4 token tiles
IK = I // P               # 8 k-tiles over intermediate
NEG1 = -1.0e30
NEG2 = -2.0e30


def _routing(tc, d, pools, xks):
    """Baseline exact-fp32 routing. Produces cw_all [128, TM, E] fp32
    (combine weights, 0 when unselected) and sel_bf [128, TM, E] bf16."""
    nc = tc.nc
    sb, wk, st, ps = pools

    gwT = sb.tile([P, HK, E], F32, name="gwT")
    nc.sync.dma_start(gwT[:], d["gwT"].rearrange("(k p) e -> p k e", p=P))
    gbb = sb.tile([P, E], F32, name="gbb")
    nc.sync.dma_start(gbb[:], d["gbb"][:])
    neg = sb.tile([P, E], F32, name="neg")
    nc.vector.memset(neg[:], NEG1)

    # logitsT [E, T] = gate_w @ x.T  (fp32, exact)
    plgT = ps.tile([E, T], F32, tag="small", bufs=2)
    for k in range(HK):
        nc.tensor.matmul(plgT[:], gwT[:, k, :], xks[k][:],
                         start=(k == 0), stop=(k == HK - 1))
    lgT = wk.tile([E, T], F32, tag="lgT", bufs=1)
    nc.vector.tensor_copy(lgT[:], plgT[:])

    ident = sb.tile([P, P], F32, name="ident")
    make_identity(nc, ident)

    cw_all = sb.tile([P, TM, E], F32, name="cw_all")
    sel_bf = sb.tile([P, TM, E], BF16, name="sel_bf")
    for m in range(TM):
        plg = ps.tile([P, E], F32, tag="small", bufs=2)
        nc.tensor.transpose(plg[:], lgT[:, m * P:(m + 1) * P], ident[:E, :E])
        s_t = wk.tile([P, E], F32, tag="s_t")
        nc.scalar.activation(s_t[:], plg[:], AF.Sigmoid)
        sc = wk.tile([P, E], F32, tag="sc")
        nc.vector.tensor_add(sc[:], s_t[:], gbb[:])

        # group score: sum of top-2 within each group of 4
        sc3 = sc[:].rearrange("p (g f) -> p g f", f=4)
        ga = wk.tile([P, NG], F32, tag="ga")
        gb_ = wk.tile([P, NG], F32, tag="gb_")
        gc = wk.tile([P, NG], F32, tag="gc")
        gd = wk.tile([P, NG], F32, tag="gd")
        nc.vector.tensor_tensor(ga[:], sc3[:, :, 0], sc3[:, :, 1], ALU.max)
        nc.vector.tensor_tensor(gb_[:], sc3[:, :, 0], sc3[:, :, 1], ALU.min)
        nc.vector.tensor_tensor(gc[:], sc3[:, :, 2], sc3[:, :, 3], ALU.max)
        nc.vector.tensor_tensor(gd[:], sc3[:, :, 2], sc3[:, :, 3], ALU.min)
        hi = wk.tile([P, NG], F32, tag="hi")
        lo = wk.tile([P, NG], F32, tag="lo")
        mid = wk.tile([P, NG], F32, tag="mid")
        nc.vector.tensor_tensor(hi[:], ga[:], gc[:], ALU.max)
        nc.vector.tensor_tensor(lo[:], ga[:], gc[:], ALU.min)
        nc.vector.tensor_tensor(mid[:], gb_[:], gd[:], ALU.max)
        gsc = wk.tile([P, NG], F32, tag="gsc")
        nc.vector.tensor_tensor(gsc[:], lo[:], mid[:], ALU.max)
        nc.vector.tensor_add(gsc[:], gsc[:], hi[:])

        # top-TG groups -> 0/1 group mask
        gm8 = wk.tile([P, 8], F32, tag="gm8")
        nc.vector.max(gm8[:], gsc[:])
        nc.vector.memset(gm8[:, TG:], NEG1)
        gz = wk.tile([P, NG], F32, tag="gz")
        nc.vector.match_replace(out=gz[:], in_to_replace=gm8[:], in_values=gsc[:], imm_value=NEG1)
        gmask = wk.tile([P, NG], mybir.dt.uint32, tag="gmask")
        nc.vector.tensor_scalar(gmask[:], gz[:], -5.0e29, None, op0=ALU.is_le)

        # expand to experts, mask scores
        emask = wk.tile([P, E], mybir.dt.uint32, tag="emask")
        em3 = emask[:].rearrange("p (g f) -> p g f", f=4)
        nc.vector.tensor_copy(em3[:], gmask[:, :, None].to_broadcast([P, NG, 4]))
        msk = wk.tile([P, E], F32, tag="msk")
        nc.vector.select(out=msk[:], mask=emask[:], on_true=sc[:], on_false=neg[:])

        # top-K experts -> 0/1 selection mask
        t8 = wk.tile([P, 8], F32, tag="t8")
        nc.vector.max(t8[:], msk[:])
        mz = wk.tile([P, E], F32, tag="mz")
        nc.vector.match_replace(out=mz[:], in_to_replace=t8[:], in_values=msk[:], imm_value=NEG2)
        sel = wk.tile([P, E], F32, tag="selm")
        nc.vector.tensor_scalar(sel[:], mz[:], -1.5e30, None, op0=ALU.is_le)
        nc.vector.tensor_copy(sel_bf[:, m, :], sel[:])

        # weights: s * sel, renormalized, * SCALE
        wr = wk.tile([P, E], F32, tag="wr")
        nc.vector.tensor_mul(wr[:], s_t[:], sel[:])
        ws = wk.tile([P, 1], F32, tag="ws")
        nc.vector.reduce_sum(ws[:], wr[:], axis=mybir.AxisListType.X)
        rec = wk.tile([P, 1], F32, tag="rec")
        nc.vector.reciprocal(rec[:], ws[:])
        coef = wk.tile([P, 1], F32, tag="coef")
        nc.vector.tensor_scalar_mul(coef[:], rec[:], SCALE)
        nc.vector.tensor_scalar_mul(cw_all[:, m, :], wr[:], coef[:])
    return cw_all, sel_bf, ident


def _dispatch_mats(tc, d, pools, cw_all, sel_bf, ident, C):
    """Build Sg [128, TM, EL, C] bf16 (gather one-hot, token-partitioned)
    and SwT/SwTr (weighted scatter, slot-partitioned [*, EL, T])."""
    nc = tc.nc
    sb, wk, st, ps = pools
    CR = C - P

    # iota constants
    io_slot_i = sb.tile([P, C], I32, name="io_slot_i")
    nc.gpsimd.iota(io_slot_i[:], pattern=[[1, C]], base=1, channel_multiplier=0)
    io_slot = sb.tile([P, C], F32, name="io_slot")
    nc.vector.tensor_copy(io_slot[:], io_slot_i[:])
    iop_i = sb.tile([P, 1], I32, name="iop_i")
    nc.gpsimd.iota(iop_i[:], pattern=[[1, 1]], base=1, channel_multiplier=1)
    iop = sb.tile([P, 1], F32, name="iop")
    nc.vector.tensor_copy(iop[:], iop_i[:])
    if CR > 0:
        iopr_i = sb.tile([P, 1], I32, name="iopr_i")
        nc.gpsimd.iota(iopr_i[:], pattern=[[1, 1]], base=P + 1, channel_multiplier=1)
        iopr = sb.tile([P, 1], F32, name="iopr")
        nc.vector.tensor_copy(iopr[:], iopr_i[:])

    ones_bf = sb.tile([P, P], BF16, name="ones_bf")
    nc.vector.memset(ones_bf[:], 1.0)
    triu_bf = sb.tile([P, P], BF16, name="triu_bf")
    make_upper_triangular(nc, triu_bf, val=1.0, diag=True)

    bsel1 = sb.tile([E, EL, P], BF16, name="bsel1")
    nc.sync.dma_start(bsel1[:], d["bsel1"][:])
    bsel2 = sb.tile([E, EL], BF16, name="bsel2")
    nc.sync.dma_start(bsel2[:], d["bsel2"][:])

    # prefix[t, e] (inclusive) for all experts, per token tile
    pfx_all = sb.tile([P, TM, E], F32, name="pfx_all")
    for m in range(TM):
        ppfx = ps.tile([P, E], F32, tag="small", bufs=2)
        for mp in range(m):
            nc.tensor.matmul(ppfx[:], ones_bf[:], sel_bf[:, mp, :],
                             start=(mp == 0), stop=False)
        nc.tensor.matmul(ppfx[:], triu_bf[:], sel_bf[:, m, :],
                         start=(m == 0), stop=True)
        nc.vector.tensor_copy(pfx_all[:, m, :], ppfx[:])

    # transposed prefix / combine weights [E, T] (bf16; both exact enough)
    pfxT = sb.tile([E, T], BF16, name="pfxT")
    cwT = sb.tile([E, T], BF16, name="cwT")
    for m in range(TM):
        tp = ps.tile([E, P], F32, tag="small", bufs=2)
        nc.tensor.transpose(tp[:], pfx_all[:, m, :], ident[:])
        nc.vector.tensor_copy(pfxT[:, m * P:(m + 1) * P], tp[:])
        tc2 = ps.tile([E, P], F32, tag="small", bufs=2)
        nc.tensor.transpose(tc2[:], cw_all[:, m, :], ident[:])
        nc.vector.tensor_copy(cwT[:, m * P:(m + 1) * P], tc2[:])

    # local-expert columns: pfxL/cwL [128 tok, EL] via extraction matmul
    Sg = sb.tile([P, TM, EL, C], BF16, name="Sg")
    for m in range(TM):
        pl = ps.tile([P, EL], F32, tag="small", bufs=2)
        nc.tensor.matmul(pl[:], pfxT[:, m * P:(m + 1) * P], bsel2[:],
                         start=True, stop=True)
        cl = ps.tile([P, EL], F32, tag="small", bufs=2)
        nc.tensor.matmul(cl[:], cwT[:, m * P:(m + 1) * P], bsel2[:],
                         start=True, stop=True)
        sl = wk.tile([P, EL], F32, tag="sl")
        nc.vector.tensor_scalar(sl[:], cl[:], 0.0, None, op0=ALU.is_gt)
        pfl = wk.tile([P, EL], F32, tag="pfl")
        nc.vector.tensor_copy(pfl[:], pl[:])
        for j in range(EL):
            nc.vector.tensor_scalar(Sg[:, m, j, :], io_slot[:],
                                    pfl[:, j:j + 1], sl[:, j:j + 1],
                                    op0=ALU.is_equal, op1=ALU.mult)

    # scatter matrices: SwT [slot<=128, EL, T], SwTr [slot 128..C, EL, T]
    SwT = sb.tile([P, EL, T], BF16, name="SwT")
    SwTr = sb.tile([P, EL, T], BF16, name="SwTr") if CR > 0 else None
    for j in range(EL):
        bp = ps.tile([P, T], F32, tag="small", bufs=2)
        nc.tensor.matmul(bp[:], bsel1[:, j, :], pfxT[:], start=True, stop=True)
        bc = ps.tile([P, T], F32, tag="small", bufs=2)
        nc.tensor.matmul(bc[:], bsel1[:, j, :], cwT[:], start=True, stop=True)
        eqt = wk.tile([P, T], F32, tag="eqt", bufs=1)
        nc.vector.tensor_scalar(eqt[:], bp[:], iop[:], None, op0=ALU.is_equal)
        nc.vector.tensor_tensor(SwT[:, j, :], eqt[:], bc[:], ALU.mult)
        if CR > 0:
            eqr = wk.tile([P, T], F32, tag="eqr", bufs=1)
            nc.vector.tensor_scalar(eqr[0:CR, :], bp[0:CR, :], iopr[0:CR, :],
                                    None, op0=ALU.is_equal)
            nc.vector.tensor_tensor(SwTr[0:CR, j, :], eqr[0:CR, :],
                                    bc[0:CR, :], ALU.mult)
    return Sg, SwT, SwTr


def _build_body(tc, d, pools, C):
    nc = tc.nc
    sb, wk, st, ps = pools
    CR = C - P

    # x loads: fp32 k-tiles (routing), bf16 transposed (shared gu),
    # bf16 token-major (gather lhsT)
    xks = []
    xr32 = d["xT"].rearrange("(k p) t -> p k t", p=P)
    for k in range(HK):
        xk = st.tile([P, T], F32, tag="x32", bufs=4, name=f"xk{k}")
        nc.sync.dma_start(xk[:], xr32[:, k, :])
        xks.append(xk)
    xTb = sb.tile([P, HK, T], BF16, name="xTb")
    nc.sync.dma_start(xTb[:], d["xTb"].rearrange("(k p) t -> p k t", p=P))
    x_tok = sb.tile([P, TM, H], BF16, name="x_tok")
    nc.sync.dma_start(x_tok[:], d["x_tok"].rearrange("(m p) h -> p m h", p=P))

    cw_all, sel_bf, ident = _routing(tc, d, pools, xks)

    # ---- shared experts gate_up (dense over all tokens) ----
    psh = [ps.tile([P, T], F32, tag="mm", name=f"psh{i}") for i in range(4)]
    for k in range(HK):
        swg = st.tile([P, 512], BF16, tag="swg", bufs=3, name="swg")
        nc.sync.dma_start(swg[:], d["swgu"][k, :, :])
        for i in range(4):
            nc.tensor.matmul(psh[i][:], swg[:, i * P:(i + 1) * P], xTb[:, k, :],
                             start=(k == 0), stop=(k == HK - 1))
    acts_sh = sb.tile([P, 2, T], BF16, name="acts_sh")
    for t in range(2):
        sgs = wk.tile([P, T], F32, tag="sgs", bufs=1)
        nc.scalar.activation(sgs[:], psh[t][:], AF.Silu)
        nc.vector.tensor_tensor(acts_sh[:, t, :], sgs[:], psh[2 + t][:], ALU.mult)

    Sg, SwT, SwTr = _dispatch_mats(tc, d, pools, cw_all, sel_bf, ident, C)

    # ---- gather: x_g [h-tile, e, slot] ----
    x_g = sb.tile([P, HK, EL, C], BF16, name="x_g")
    for h in range(HK):
        pg = [ps.tile([P, C], F32, tag="mm", name=f"pg{e}") for e in range(EL)]
        for k in range(TM):
            for e in range(EL):
                nc.tensor.matmul(pg[e][:], x_tok[:, k, h * P:(h + 1) * P],
                                 Sg[:, k, e, :], start=(k == 0), stop=(k == TM - 1))
        for e in range(EL):
            nc.vector.tensor_copy(x_g[:, h, e, :], pg[e][:])

    # ---- routed gate_up + silu: act [i-tile, e, slot] ----
    act = sb.tile([P, EL, IK, C], BF16, name="act")
    for e in range(EL):
        sgw = [None, None]
        for cg in (0, 2, 1, 3):
            wg = st.tile([P, 8, 512], BF16, tag="wgu", bufs=2, name="wg")
            nc.sync.dma_start(
                wg[:], d["wgu"][e, cg, 0:8, :, :].rearrange("k p f -> p k f"))
            wg2 = st.tile([P, 8, 512], BF16, tag="wgu", bufs=2, name="wg2")
            nc.sync.dma_start(
                wg2[:], d["wgu"][e, cg, 8:16, :, :].rearrange("k p f -> p k f"))
            pgu = [ps.tile([P, C], F32, tag="mm", name=f"pgu{i}") for i in range(4)]
            for k in range(HK):
                wt = wg if k < 8 else wg2
                for i in range(4):
                    nc.tensor.matmul(pgu[i][:], wt[:, k % 8, i * P:(i + 1) * P],
                                     x_g[:, k, e, :], start=(k == 0),
                                     stop=(k == HK - 1))
            half = cg % 2  # 0 -> i-tiles 0..3, 1 -> i-tiles 4..7
            if cg < 2:  # gate columns -> silu
                sgt = wk.tile([P, 4, C], F32, tag=f"sg{half}", bufs=1, name=f"sgt{half}")
                sgw[half] = sgt
                for i in range(4):
                    nc.scalar.activation(sgt[:, i, :], pgu[i][:], AF.Silu)
            else:       # up columns -> multiply
                for i in range(4):
                    nc.vector.tensor_tensor(act[:, e, 4 * half + i, :],
                                            sgw[half][:, i, :], pgu[i][:], ALU.mult)

    # ---- down (token-partitioned): y [slot, h] per expert ----
    y = sb.tile([P, EL, H], BF16, name="y")
    yr = sb.tile([P, EL, H], BF16, name="yr") if CR > 0 else None
    for e in range(EL):
        for hf in range(2):
            ya = [ps.tile([P, 512], F32, tag="mm", name=f"ya{n}") for n in range(2)]
            yb = [ps.tile([P, 512], F32, tag="mm", name=f"yb{n}") for n in range(2)] \
                if CR > 0 else []
            for q in range(2):
                wdt = st.tile([P, 4, 1024], BF16, tag="wd", bufs=3, name="wdt")
                nc.sync.dma_start(
                    wdt[:], d["wd"][e, hf, 4 * q:4 * q + 4, :, :]
                    .rearrange("k p f -> p k f"))
                for kk in range(4):
                    i2 = 4 * q + kk
                    st_, sp_ = (i2 == 0), (i2 == IK - 1)
                    for n in range(2):
                        nc.tensor.matmul(ya[n][:], act[:, e, i2, 0:P],
                                         wdt[:, kk, 512 * n:512 * (n + 1)],
                                         start=st_, stop=sp_)
                    for n in range(2):
                        if CR > 0:
                            nc.tensor.matmul(yb[n][0:CR, :], act[:, e, i2, P:C],
                                             wdt[:, kk, 512 * n:512 * (n + 1)],
                                             start=st_, stop=sp_)
            for n in range(2):
                nc.vector.tensor_copy(y[:, e, 1024 * hf + 512 * n:
                                        1024 * hf + 512 * (n + 1)], ya[n][:])
                if CR > 0:
                    nc.vector.tensor_copy(yr[0:CR, e, 1024 * hf + 512 * n:
                                             1024 * hf + 512 * (n + 1)],
                                          yb[n][0:CR, :])

    # ---- scatter + shared down: out [t-tile, H] ----
    swd_sb = sb.tile([P, 2, H], BF16, name="swd_sb")
    nc.sync.dma_start(swd_sb[:], d["swd"].rearrange("k p f -> p k f"))
    n_contrib = EL * (2 if CR > 0 else 1) + 2
    for tc_ in range(TM):
        po = [ps.tile([P, 512], F32, tag="mm", name=f"po{n}") for n in range(4)]
        ci = 0
        for e in range(EL):
            lhs = SwT[:, e, tc_ * P:(tc_ + 1) * P]
            for n in range(4):
                nc.tensor.matmul(po[n][:], lhs, y[:, e, 512 * n:512 * (n + 1)],
                                 start=(ci == 0), stop=(ci == n_contrib - 1))
            ci += 1
            if CR > 0:
                lhsr = SwTr[0:CR, e, tc_ * P:(tc_ + 1) * P]
                for n in range(4):
                    nc.tensor.matmul(po[n][:], lhsr,
                                     yr[0:CR, e, 512 * n:512 * (n + 1)],
                                     start=(ci == 0), stop=(ci == n_contrib - 1))
                ci += 1
        for kk in range(2):
            lhs = acts_sh[:, kk, tc_ * P:(tc_ + 1) * P]
            for n in range(4):
                nc.tensor.matmul(po[n][:], lhs, swd_sb[:, kk, 512 * n:512 * (n + 1)],
                                 start=(ci == 0), stop=(ci == n_contrib - 1))
            ci += 1
        for n in range(4):
            ot = wk.tile([P, 512], F32, tag="ost", bufs=4, name="ot")
            nc.vector.tensor_copy(ot[:], po[n][:])
            nc.sync.dma_start(d["out"][tc_, :, 512 * n:512 * (n + 1)], ot[:])


def build_nc(C, repeat=1):
    nc = bacc.Bacc("TRN2", target_bir_lowering=False, debug=False, num_devices=NCORES)
    d = {
        "xT": nc.dram_tensor("xT", [H, T], F32, kind="ExternalInput").ap(),
        "xTb": nc.dram_tensor("xTb", [H, T], BF16, kind="ExternalInput").ap(),
        "x_tok": nc.dram_tensor("x_tok", [T, H], BF16, kind="ExternalInput").ap(),
        "gwT": nc.dram_tensor("gwT", [H, E], F32, kind="ExternalInput").ap(),
        "gbb": nc.dram_tensor("gbb", [P, E], F32, kind="ExternalInput").ap(),
        "bsel1": nc.dram_tensor("bsel1", [E, EL, P], BF16, kind="ExternalInput").ap(),
        "bsel2": nc.dram_tensor("bsel2", [E, EL], BF16, kind="ExternalInput").ap(),
        "wgu": nc.dram_tensor("wgu", [EL, 4, HK, P, 512], BF16, kind="ExternalInput").ap(),
        "wd": nc.dram_tensor("wd", [EL, 2, IK, P, 1024], BF16, kind="ExternalInput").ap(),
        "swgu": nc.dram_tensor("swgu", [HK, P, 512], BF16, kind="ExternalInput").ap(),
        "swd": nc.dram_tensor("swd", [2, P, H], BF16, kind="ExternalInput").ap(),
        "out": nc.dram_tensor("out", [TM, P, H], F32, kind="ExternalOutput").ap(),
    }
    with tile.TileContext(nc) as tc:
        with (
            tc.tile_pool(name="sb", bufs=1) as sb,
            tc.tile_pool(name="wk", bufs=2) as wk,
            tc.tile_pool(name="st", bufs=3) as st,
            tc.tile_pool(name="ps", bufs=6, space="PSUM") as ps,
        ):
            pools = (sb, wk, st, ps)
            if repeat == 1:
                _build_body(tc, d, pools, C)
            else:
                with tc.For_i(0, repeat, 1):
                    _build_body(tc, d, pools, C)
    nc.compile()
    return nc


def derive_capacity(hidden_states, gate_w, gate_bias):
    """Exact numpy replica of the routing to size the per-expert capacity."""
    x = np.asarray(hidden_states, np.float32)
    gw = np.asarray(gate_w, np.float32)
    gb = np.asarray(gate_bias, np.float32)
    logits = x @ gw.T
    s = 1.0 / (1.0 + np.exp(-logits))
    sc = s + gb[None, :]
    t = sc.shape[0]
    grp = sc.reshape(t, NG, E // NG)
    gscore = np.sort(grp, -1)[:, :, -2:].sum(-1)
    gidx = np.argsort(-gscore, -1)[:, :TG]
    gmask = np.zeros((t, NG), bool)
    gmask[np.arange(t)[:, None], gidx] = True
    emask = np.repeat(gmask, E // NG, 1)
    masked = np.where(emask, sc, -np.inf)
    ids = np.argsort(-masked, -1)[:, :K]
    selm = np.zeros((t, E), bool)
    selm[np.arange(t)[:, None], ids] = True
    counts = selm.sum(0)
    # +2 safety margin for host/device near-tie divergence, round up to 8
    c = int(counts.max()) + 2
    c = max(136, (c + 7) // 8 * 8)
    return c


def shard_inputs(hidden_states, gate_w, gate_bias, w_gate_up, w_down,
                 shared_w_gate_up, shared_w_down):
    bf = ml_dtypes.bfloat16
    x = np.ascontiguousarray(hidden_states, dtype=np.float32)
    xT = np.ascontiguousarray(x.T)
    xTb = np.ascontiguousarray(xT.astype(bf))
    x_tok = np.ascontiguousarray(x.astype(bf))
    gwT = np.ascontiguousarray(np.asarray(gate_w, np.float32).T)
    gbb = np.ascontiguousarray(np.tile(np.asarray(gate_bias, np.float32)[None, :], (P, 1)))
    wgu = np.asarray(w_gate_up, np.float32)
    wd = np.asarray(w_down, np.float32)
    swgu = np.asarray(shared_w_gate_up, np.float32)
    swd = np.asarray(shared_w_down, np.float32)

    in_maps = []
    for c in range(NCORES):
        bsel1 = np.zeros((E, EL, P), dtype=bf)
        bsel2 = np.zeros((E, EL), dtype=bf)
        for j in range(EL):
            bsel1[EL * c + j, j, :] = 1.0
            bsel2[EL * c + j, j] = 1.0
        # wgu_r[e, cg, k] = wgu[4c+e][k*128:(k+1)*128, cg*512:(cg+1)*512]
        wgu_c = wgu[EL * c:EL * (c + 1)].astype(bf)
        wgu_r = np.ascontiguousarray(
            wgu_c.reshape(EL, HK, P, 4, 512).transpose(0, 3, 1, 2, 4))
        # wd_r[e, hf, i2] = wd[4c+e][i2*128:(i2+1)*128, hf*1024:(hf+1)*1024]
        wd_c = wd[EL * c:EL * (c + 1)].astype(bf)
        wd_r = np.ascontiguousarray(
            wd_c.reshape(EL, IK, P, 2, 1024).transpose(0, 3, 1, 2, 4))
        # shared: pack [gate slice | up slice] columns -> [H, 512]
        sw = np.concatenate([
            swgu[:, c * SI:(c + 1) * SI],
            swgu[:, 2 * I + c * SI: 2 * I + (c + 1) * SI],
        ], axis=1).astype(bf)
        swgu_r = np.ascontiguousarray(sw.reshape(HK, P, 512))
        swd_r = np.ascontiguousarray(
            swd[c * SI:(c + 1) * SI, :].astype(bf).reshape(2, P, H))
        in_maps.append({
            "xT": xT, "xTb": xTb, "x_tok": x_tok, "gwT": gwT, "gbb": gbb,
            "bsel1": bsel1, "bsel2": bsel2,
            "wgu": wgu_r, "wd": wd_r, "swgu": swgu_r, "swd": swd_r,
        })
    return in_maps


def combine(results):
    """results: list of per-core {"out": [TM, P, H] fp32}; returns [T, H]."""
    acc = np.zeros((TM, P, H), dtype=np.float32)
    for r in results:
        acc += r["out"]
    return np.ascontiguousarray(acc.reshape(T, H))


_NC_CACHE = {}


def kernel(hidden_states, gate_w, gate_bias, w_gate_up, w_down,
           shared_w_gate_up, shared_w_down):
    C = derive_capacity(hidden_states, gate_w, gate_bias)
    if ("nc", C) not in _NC_CACHE:
        _NC_CACHE[("nc", C)] = build_nc(C, repeat=1)
    nc = _NC_CACHE[("nc", C)]
    in_maps = shard_inputs(hidden_states, gate_w, gate_bias, w_gate_up, w_down,
                           shared_w_gate_up, shared_w_down)
    res = run_bass_kernel_spmd(nc, in_maps, list(range(NCORES)))
    return combine([res.results[c] for c in range(NCORES)])
